# revision 11
# baseline (speedup 1.0000x reference)
"""ConvLSTM3D encoder kernel for 8 trn2 NeuronCores (v2, bf16).

Sharding: core c in [0,8) handles batch b = c//4, z-slab k = c%4 (8 output
planes z in [8k, 8k+8)).  The sequential T=10 loop runs on-device; per-step
halo exchange (1 plane each side of the slab) is an AllGather over the 4
cores of each batch group (bf16 payload, double-buffered DRAM).

Conv mapping: gates = Wx (x) x_t (stride 2) + Wh (x) h + b is one K=128
bf16 matmul accumulation stream per output plane (N=1024 = full 32x32):
  partitions  0..95  : three z-shifted copies of h (dz = 0,1,2)
  partitions 96..122 : host-precomputed im2col taps of x_t (27 taps)
  partition  123     : ones (bias row, memset once)
For each (dy,dx) in 3x3, one matmul with an AP offset of (dy,dx) into the
padded (34x34) plane layout contracts channels x dz at once; the x-conv and
bias ride in the delta=(0,0) matmul only (their lhsT rows are zero in the
other eight).

Elementwise LSTM math runs on [32, span] slices straight out of the gates
tile (i/f/o/g live on partition quadrants 0:32/32:64/64:96/96:128 - the DVE
crossbar allows different quadrant bases per operand at nch=32), cell state
is fp16 for the 2-byte DVE fast mode.  h is written once (strided, bf16)
into the dz=1 partition group of the next h-stack; the dz=0/2 groups are
produced by two large contiguous SBUF->SBUF DMAs with a +-1 plane offset.
Boundary planes (0,7) are computed first each step so the halo collective
overlaps the interior-plane compute.
"""

import os
import sys
from contextlib import ExitStack

import numpy as np
import ml_dtypes

for _p in ("/opt/trn_rl_repo", "/root/.axon_site/_ro/trn_rl_repo"):
    if os.path.isdir(_p) and _p not in sys.path:
        sys.path.insert(0, _p)

import concourse.bass as bass
import concourse.bacc as bacc
import concourse.mybir as mybir
from concourse import tile
from concourse.bass_utils import run_bass_kernel_spmd

F32 = mybir.dt.float32
F16 = mybir.dt.float16
BF = mybir.dt.bfloat16
I32 = mybir.dt.int32

T = 10
CH = 32          # hidden channels
SLAB = 8         # output planes per core
PLW = 34         # padded plane width
PL = PLW * PLW   # padded plane elements (1156)
HS_FREE = SLAB * PL  # h-stack free size per partition (9248)
DELTAS = [(dy, dx) for dy in range(3) for dx in range(3)]
# plane processing order: boundary planes first (their h feeds the
# collective), interior after (overlaps the collective in flight)
PO = [0, 7, 1, 2, 3, 4, 5, 6]
SPAN = {p: i * 1024 for i, p in enumerate(PO)}
RG = [[0, 1, 2, 3], [4, 5, 6, 7]]
NPBF = ml_dtypes.bfloat16

_prog_cache = {}


def _build_program(nsteps=T):
    key = nsteps
    if key in _prog_cache:
        return _prog_cache[key]

    nc = bacc.Bacc(num_devices=8)

    xim_d = nc.dram_tensor("xim", [T, 27, HS_FREE], BF, kind="ExternalInput")
    whl_d = nc.dram_tensor("whl", [9, 128, 128], BF, kind="ExternalInput")
    hoff_d = nc.dram_tensor("hoff", [1, 2], I32, kind="ExternalInput")
    ones_d = nc.dram_tensor("ones", [1, HS_FREE], BF, kind="ExternalInput")
    hout_d = nc.dram_tensor("hout", [CH, SLAB, 32, 32], F32, kind="ExternalOutput")
    agin = [nc.dram_tensor(f"agin{i}", [3, CH, 1024], BF) for i in range(2)]
    agout = [nc.dram_tensor(f"agout{i}", [12, CH, 1024], BF)
             for i in range(2)]

    with ExitStack() as ctx:
        tc = ctx.enter_context(tile.TileContext(nc))
        pers = ctx.enter_context(tc.tile_pool(name="pers", bufs=1))
        psum = ctx.enter_context(tc.tile_pool(name="psum", bufs=4, space="PSUM"))
        work = ctx.enter_context(tc.tile_pool(name="work", bufs=2))

        hstack = [
            pers.tile([128, HS_FREE], BF, tag="hstackA", name="hstackA"),
            pers.tile([128, HS_FREE], BF, tag="hstackB", name="hstackB"),
        ]
        wh_sb = pers.tile([128, 9 * 128], BF, tag="wh")
        # elementwise operand placement: every two-input DVE op needs both
        # inputs on the same base partition, so: g at base 0 (own tile),
        # c/prod/tmp at base 32, tanh(c) at base 64 (next to o).
        c_state = pers.tile([64, 8 * 1024], F16, tag="cstate")
        prod = pers.tile([64, 8 * 1024], F16, tag="prod")
        tmp = pers.tile([64, 8 * 1024], F16, tag="tmp")
        tanhc = pers.tile([96, 8 * 1024], F16, tag="tanhc")
        hfin = pers.tile([32, 8 * 1024], F32, tag="hfin")
        zscr = pers.tile([CH, 1024], BF, tag="zscr")

        # ---- init ----
        nc.vector.memset(hstack[0][:, :], 0.0)
        nc.gpsimd.memset(hstack[1][:, :], 0.0)
        nc.vector.memset(c_state[32:64, :], 0.0)
        nc.vector.memset(zscr[:, :], 0.0)
        for i in range(2):
            nc.sync.dma_start(out=hstack[i][123:124, :], in_=ones_d[:, :])
        for i in range(2):
            nc.sync.dma_start(out=agin[i][2], in_=zscr[:, :])
        for _d in range(9):
            nc.sync.dma_start(out=wh_sb[:, 128 * _d:128 * (_d + 1)],
                              in_=whl_d[_d])
        nc.sync.dma_start(out=hstack[0][96:123, :], in_=xim_d[0])

        r_lo = nc.alloc_register(mybir.EngineType.Pool, "r_lo")
        r_hi = nc.alloc_register(mybir.EngineType.Pool, "r_hi")
        nc.reg_load(r_lo, hoff_d[0:1, 0:1])
        nc.reg_load(r_hi, hoff_d[0:1, 1:2])
        rv_lo = nc.snap(r_lo, min_val=0, max_val=11)
        rv_hi = nc.snap(r_hi, min_val=0, max_val=11)

        hsv = [h[:, :].rearrange("p (z y x) -> p z y x", z=SLAB, y=PLW, x=PLW)
               for h in hstack]

        def emit_plane(t, curv, gates, p):
            """9-delta matmul accumulation + gate activations for plane p."""
            ps = psum.tile([128, 1024], F32, tag="ps", name="ps")
            if t == 0:
                for h in range(2):
                    nc.tensor.matmul(ps[:, 512 * h:512 * (h + 1)],
                                     lhsT=wh_sb[:, 0:128],
                                     rhs=curv[:, p, 16 * h:16 * h + 16, 0:32],
                                     start=True, stop=True)
            else:
                for di, (dy, dx) in enumerate(DELTAS):
                    for h in range(2):
                        nc.tensor.matmul(
                            ps[:, 512 * h:512 * (h + 1)],
                            lhsT=wh_sb[:, 128 * di:128 * (di + 1)],
                            rhs=curv[:, p, 16 * h + dy:16 * h + dy + 16,
                                     dx:dx + 32],
                            start=(di == 0), stop=(di == 8))
            s = SPAN[p]
            nc.scalar.activation(gates[0:96, s:s + 1024], ps[0:96, :],
                                 mybir.ActivationFunctionType.Sigmoid)
            nc.scalar.activation(g_t[0:32, s:s + 1024], ps[96:128, :],
                                 mybir.ActivationFunctionType.Tanh)

        def emit_group(t, gates, g_t, nxtv, planes, s0, s1):
            """LSTM elementwise update for gate span [s0:s1] (planes list)."""
            i_sl = gates[0:32, s0:s1]
            f_sl = gates[32:64, s0:s1]
            o_sl = gates[64:96, s0:s1]
            c_sl = c_state[32:64, s0:s1]
            nc.vector.tensor_mul(prod[32:64, s0:s1], i_sl, g_t[0:32, s0:s1])
            nc.vector.tensor_mul(tmp[32:64, s0:s1], f_sl, c_sl)
            nc.vector.tensor_add(c_sl, tmp[32:64, s0:s1], prod[32:64, s0:s1])
            nc.scalar.activation(tanhc[64:96, s0:s1], c_sl,
                                 mybir.ActivationFunctionType.Tanh)
            last = t == nsteps - 1
            for pl, a, b in planes:
                o_ap = o_sl[:, a - s0:b - s0].rearrange(
                    "p (z y x) -> p z y x", z=(b - a) // 1024, y=32, x=32)
                t_ap = tanhc[64:96, a:b].rearrange(
                    "p (z y x) -> p z y x", z=(b - a) // 1024, y=32, x=32)
                if last:
                    nc.vector.tensor_mul(
                        hfin[:, a:b].rearrange("p (z y x) -> p z y x",
                                               z=(b - a) // 1024, y=32, x=32),
                        o_ap, t_ap)
                else:
                    npl = (b - a) // 1024
                    nc.vector.tensor_mul(
                        nxtv[32:64, pl:pl + npl, 1:33, 1:33], o_ap, t_ap)

        T_ = nsteps
        for t in range(T_):
            cur, nxt = hstack[t % 2], hstack[(t + 1) % 2]
            curv, nxtv = hsv[t % 2], hsv[(t + 1) % 2]
            last = t == T_ - 1
            gates = work.tile([96, 8 * 1024], BF, tag="gates", name="gates")
            g_t = work.tile([32, 8 * 1024], BF, tag="g_t", name="g_t")
            if not last:
                nc.sync.dma_start(out=nxt[96:123, :], in_=xim_d[t + 1])

            # boundary planes first; their h feeds this step's collective
            emit_plane(t, curv, gates, 0)
            emit_plane(t, curv, gates, 7)
            emit_group(t, gates, g_t, nxtv, [(0, 0, 1024), (7, 1024, 2048)],
                       0, 2048)
            if not last:
                ag_i, ag_o = agin[t % 2], agout[t % 2]
                nc.sync.dma_start(
                    out=ag_i[0].rearrange("c (y x) -> c y x", y=32, x=32),
                    in_=nxtv[32:64, 0, 1:33, 1:33])
                nc.sync.dma_start(
                    out=ag_i[1].rearrange("c (y x) -> c y x", y=32, x=32),
                    in_=nxtv[32:64, 7, 1:33, 1:33])
                nc.gpsimd.collective_compute(
                    "AllGather", mybir.AluOpType.bypass, replica_groups=RG,
                    ins=[ag_i[:, :, :]], outs=[ag_o[:, :, :]])
                # g2 slot 6 <- h7 available right after the boundary group
                nc.scalar.dma_start(out=nxtv[64:96, 6:7, :, :],
                                    in_=nxtv[32:64, 7:8, :, :])

            # interior planes in pairs (chunked elementwise + dz copies so
            # next-step matmuls wait only on per-plane producers)
            for p0 in (1, 3, 5):
                emit_plane(t, curv, gates, p0)
                emit_plane(t, curv, gates, p0 + 1)
                s0 = SPAN[p0]
                emit_group(t, gates, g_t, nxtv, [(p0, s0, s0 + 2048)],
                           s0, s0 + 2048)
                if not last:
                    # g0 slot p <- h[p-1]; g2 slot p <- h[p+1]
                    hi = p0 + 2 if p0 < 5 else p0 + 3  # fold g0[7]<-h6 into last pair
                    nc.scalar.dma_start(out=nxtv[0:32, p0:hi, :, :],
                                        in_=nxtv[32:64, p0 - 1:hi - 1, :, :])
                    nc.scalar.dma_start(out=nxtv[64:96, p0 - 1:p0 + 1, :, :],
                                        in_=nxtv[32:64, p0:p0 + 2, :, :])

            if not last:
                halo_lo = ag_o[bass.ds(rv_lo, 1)].squeeze(0).rearrange(
                    "c (y x) -> c y x", y=32, x=32)
                halo_hi = ag_o[bass.ds(rv_hi, 1)].squeeze(0).rearrange(
                    "c (y x) -> c y x", y=32, x=32)
                nc.gpsimd.dma_start(out=nxtv[0:32, 0, 1:33, 1:33], in_=halo_lo)
                nc.gpsimd.dma_start(out=nxtv[64:96, 7, 1:33, 1:33], in_=halo_hi)
            else:
                for pl in range(SLAB):
                    s = SPAN[pl]
                    nc.sync.dma_start(
                        out=hout_d[:, pl, :, :],
                        in_=hfin[:, s:s + 1024].rearrange(
                            "c (y x) -> c y x", y=32, x=32))

    nc.finalize()
    _prog_cache[key] = nc
    return nc


def _host_inputs(input_batch, Wx, Wh, b):
    input_batch = np.asarray(input_batch, dtype=np.float32)
    Wx = np.asarray(Wx, dtype=np.float32)
    Wh = np.asarray(Wh, dtype=np.float32)
    b = np.asarray(b, dtype=np.float32)

    xp = np.zeros((2, T, 66, 66, 66), np.float32)
    xp[:, :, 1:65, 1:65, 1:65] = input_batch[:, :, 0]

    whl = np.zeros((9, 128, 128), np.float32)
    for di, (dy, dx) in enumerate(DELTAS):
        for g in range(3):
            whl[di, 32 * g:32 * g + 32, :] = Wh[:, :, g, dy, dx].T
    whl[0, 96:123, :] = Wx[:, 0].reshape(128, 27).T
    whl[0, 123, :] = b
    whl = whl.astype(NPBF)

    in_maps = []
    for c in range(8):
        bidx, k = divmod(c, 4)
        z0 = 8 * k
        xim = np.zeros((T, 27, SLAB, PLW, PLW), np.float32)
        for tz in range(3):
            for ty in range(3):
                for tx in range(3):
                    tap = tz * 9 + ty * 3 + tx
                    xim[:, tap, :, 0:32, 0:32] = xp[
                        bidx, :, 2 * z0 + tz:2 * z0 + tz + 16:2,
                        ty:ty + 64:2, tx:tx + 64:2]
        lo_slot = 3 * k + 2 if k == 0 else 3 * (k - 1) + 1
        hi_slot = 3 * k + 2 if k == 3 else 3 * (k + 1)
        in_maps.append({
            "xim": xim.reshape(T, 27, HS_FREE).astype(NPBF),
            "whl": whl,
            "ones": np.ones((1, HS_FREE), NPBF),
            "hoff": np.array([[lo_slot, hi_slot]], np.int32),
        })
    return in_maps


def run_cores(in_maps, nsteps=T, **kwargs):
    nc = _build_program(nsteps)
    return run_bass_kernel_spmd(nc, in_maps, list(range(8)), **kwargs)


def kernel(input_batch, Wx, Wh, b):
    in_maps = _host_inputs(input_batch, Wx, Wh, b)
    res = run_cores(in_maps)
    out = np.zeros((2, CH, 32, 32, 32), np.float32)
    for c in range(8):
        bidx, k = divmod(c, 4)
        out[bidx, :, 8 * k:8 * k + 8] = res.results[c]["hout"]
    return out
